# revision 13
# baseline (speedup 1.0000x reference)
"""Trainium2 Bass kernel for nn_DiffusionCNN (submanifold sparse 3x3x3 CNN).

Valid-pair design (8-core SPMD, no collectives):
  - The 27-offset rulebook is ~91% empty (uniform 9.5% occupancy); the
    baseline gathered every slot (1.45M rows/core) and was bound by SWDGE
    descriptor generation on the GpSimd engine (~7.8ns/idx, 11.6ms busy).
  - This kernel gathers only valid neighbor pairs, grouped per output tile
    (512 rows) and per non-center offset k as fixed-width runs (W_RUN=78
    covers the global max run of 73).  Per tile: 26*78 -> 2048 gather slots
    (incl. pads to zero rows), i.e. ~7x fewer descriptors.
  - Per k-run a single matmul (lhsT=W_k) produces channel-major
    contributions in PSUM; PE transposes flip them pair-major; the
    pair->output routing runs as a segment-sum matmul against 0/1 selection
    matrices built on the DVE (iota fp16 is_equal per-partition segids).
    Accumulation stays in fp32 PSUM.  The center offset (k=13, identity)
    is applied densely (one matmul from a channel-major x/h1 slice).
  - h1 window is fixed at [s-2048, s-2048+29184) for every core so the
    shared program uses static offsets; per-core variation lives entirely
    in the index/segid streams.
  - All matmuls bf16 with fp32 PSUM accumulation.

Host-side work: slicing inputs, building per-core gather index + segid
streams (int16/fp16), packing weights, re-assembling the output.
"""

import numpy as np
import ml_dtypes

# ---------------------------------------------------------------- constants
N = 200000
PER = 25000
NCORES = 8
C = 128
K = 27
TEMB = 6
IN_CH = 7  # features(1) + sin/cos(6)

TILE = 512
D_HALO = 2048            # fixed h1-window offset (512-aligned, >= max halo 1619)
NT1 = 57                 # h1 window tiles per core
NT2 = 49                 # output tiles per core
M_H1 = NT1 * TILE        # 29184 h1 rows computed per core
M_OUT = NT2 * TILE       # 25088 output rows per core (25000 + pad)
NZ = 1024                # zero rows at the front of each gather table
XT = 32384               # x table rows (NZ + up to 31350 real, max idx 32373)
H1T = NZ + M_H1          # 30208 h1 table rows (max idx 30207)

# k=13 is the center (identity); k=12/14 are (0,0,-1)/(0,0,+1) whose valid
# neighbors are always the adjacent table rows (z-sorted order) -> handled
# densely via shifted channel-major slices times a 0/1 mask.
KS = [k for k in range(K) if k not in (12, 13, 14)]
NK = 24
W_RUN = 73               # run slots per (tile, k); global max run is 73
SLOTS = 1792             # 24*73=1752 run slots + 40 tail pads; 14 chunks of 128
NCHK = SLOTS // 128      # 14
IDXC = SLOTS // 16       # idx columns per tile (112)
CMW = M_H1 + 2           # ch-major tables get a zero border column each side

_bf16 = ml_dtypes.bfloat16
_f16 = np.float16
SEG_DUMP = 999.0         # segid for pad slots; iota is 0..511 so never equal


# ------------------------------------------------------------- device program
def _build_program(bench_reps=0):
    import concourse.bass as bass
    import concourse.mybir as mybir
    import concourse.tile as tile
    from concourse import bacc
    from concourse.masks import make_identity

    bf = mybir.dt.bfloat16
    f32 = mybir.dt.float32
    f16 = mybir.dt.float16
    i16 = mybir.dt.int16
    AF = mybir.ActivationFunctionType

    nc = bacc.Bacc("TRN2", target_bir_lowering=False, debug=False)

    x_tab = nc.dram_tensor("x_tab", [XT, C], bf, kind="ExternalInput")
    x_cm = nc.dram_tensor("x_cm", [16, CMW], bf, kind="ExternalInput")
    i1 = nc.dram_tensor("i1", [128, NT1 * IDXC], i16, kind="ExternalInput")
    s1 = nc.dram_tensor("s1", [128, NT1 * NCHK * TILE], bf, kind="ExternalInput")
    i2 = nc.dram_tensor("i2", [128, NT2 * IDXC], i16, kind="ExternalInput")
    s2 = nc.dram_tensor("s2", [128, NT2 * NCHK * TILE], bf, kind="ExternalInput")
    w1 = nc.dram_tensor("w1", [C, NK * C], bf, kind="ExternalInput")
    w1c = nc.dram_tensor("w1c", [16, C], bf, kind="ExternalInput")
    w1z = nc.dram_tensor("w1z", [16, 2 * C], bf, kind="ExternalInput")
    w2 = nc.dram_tensor("w2", [C, NK * C], bf, kind="ExternalInput")
    w2c = nc.dram_tensor("w2c", [C, C], bf, kind="ExternalInput")
    w2z = nc.dram_tensor("w2z", [C, 2 * C], bf, kind="ExternalInput")
    m1 = nc.dram_tensor("m1", [128, NT1 * 2 * TILE], bf, kind="ExternalInput")
    m2 = nc.dram_tensor("m2", [128, NT2 * 2 * TILE], bf, kind="ExternalInput")
    w3 = nc.dram_tensor("w3", [C, C], bf, kind="ExternalInput")
    w4 = nc.dram_tensor("w4", [C, 16], bf, kind="ExternalInput")
    b1 = nc.dram_tensor("b1", [C, 1], f32, kind="ExternalInput")
    b2 = nc.dram_tensor("b2", [C, 1], f32, kind="ExternalInput")
    b3 = nc.dram_tensor("b3", [C, 1], f32, kind="ExternalInput")
    b4 = nc.dram_tensor("b4", [1, 1], f32, kind="ExternalInput")
    outd = nc.dram_tensor("out", [M_OUT], f32, kind="ExternalOutput")
    h1tab = nc.dram_tensor("h1_tab", [H1T, C], bf, kind="Internal")
    h1cm = nc.dram_tensor("h1_cm", [128, CMW], bf, kind="Internal")

    with tile.TileContext(nc) as tc:
        with (
            tc.tile_pool(name="const", bufs=1) as constp,
            tc.tile_pool(name="idx", bufs=3) as idxp,
            tc.tile_pool(name="seg", bufs=3) as segp,
            tc.tile_pool(name="ctr", bufs=3) as ctrp,
            tc.tile_pool(name="gat", bufs=4) as gatp,
            tc.tile_pool(name="csb", bufs=3) as csbp,
            tc.tile_pool(name="cpm", bufs=3) as cpmp,
            tc.tile_pool(name="sel", bufs=3) as selp,
            tc.tile_pool(name="sS", bufs=3) as ssp,
            tc.tile_pool(name="act", bufs=3) as actp,
            tc.tile_pool(name="stage", bufs=2) as stagep,
            tc.tile_pool(name="osb", bufs=2) as outp,
            tc.tile_pool(name="cps", bufs=1, space="PSUM") as cps,
            tc.tile_pool(name="ptp", bufs=1, space="PSUM") as ptp,
            tc.tile_pool(name="ops", bufs=1, space="PSUM") as opsp,
            tc.tile_pool(name="mps", bufs=1, space="PSUM") as mpsp,
        ):
            w1_sb = constp.tile([C, NK * C], bf, tag="w1")
            nc.sync.dma_start(w1_sb[:], w1[:])
            w1c_sb = constp.tile([16, C], bf, tag="w1c")
            nc.sync.dma_start(w1c_sb[:], w1c[:])
            w1z_sb = constp.tile([16, 2 * C], bf, tag="w1z")
            nc.sync.dma_start(w1z_sb[:], w1z[:])
            w2_sb = constp.tile([C, NK * C], bf, tag="w2")
            nc.sync.dma_start(w2_sb[:], w2[:])
            w2c_sb = constp.tile([C, C], bf, tag="w2c")
            nc.sync.dma_start(w2c_sb[:], w2c[:])
            w2z_sb = constp.tile([C, 2 * C], bf, tag="w2z")
            nc.sync.dma_start(w2z_sb[:], w2z[:])
            w3_sb = constp.tile([C, C], bf, tag="w3")
            nc.sync.dma_start(w3_sb[:], w3[:])
            w4_sb = constp.tile([C, 16], bf, tag="w4")
            nc.sync.dma_start(w4_sb[:], w4[:])
            b1_sb = constp.tile([C, 1], f32, tag="b1")
            nc.sync.dma_start(b1_sb[:], b1[:])
            b2_sb = constp.tile([C, 1], f32, tag="b2")
            nc.sync.dma_start(b2_sb[:], b2[:])
            b3_sb = constp.tile([C, 1], f32, tag="b3")
            nc.sync.dma_start(b3_sb[:], b3[:])
            b4_sb = constp.tile([1, 1], f32, tag="b4")
            nc.sync.dma_start(b4_sb[:], b4[:])
            ident = constp.tile([C, C], bf, tag="ident")
            make_identity(nc, ident[:])
            zblk = constp.tile([C, NZ // 128 * C], bf, tag="zblk")
            nc.vector.memset(zblk[:], 0.0)
            nc.sync.dma_start(
                h1tab[0:NZ, :].rearrange("(p c) e -> p (c e)", c=NZ // 128),
                zblk[:])

            ztiny = constp.tile([128, 1], bf, tag="ztiny")
            nc.vector.memset(ztiny[:], 0.0)
            nc.sync.dma_start(h1cm[:, 0:1], ztiny[:])
            nc.sync.dma_start(h1cm[:, CMW - 1:CMW], ztiny[:])


            def conv_tile(src_tab, idx_dram, seg_dram, w_sb, wc_sb, wz_sb,
                          cw, mk, t):
                it = idxp.tile([128, IDXC], mybir.dt.int16, tag="it")
                nc.sync.dma_start(
                    it[:], idx_dram[:, t * IDXC:(t + 1) * IDXC])
                g = gatp.tile([128, SLOTS], bf, tag="g")
                nc.gpsimd.dma_gather(
                    out_ap=g[:].rearrange("p (o n) -> p o n", o=1),
                    in_ap=src_tab[:, :],
                    idxs_ap=it[:, :],
                    num_idxs=SLOTS,
                    num_idxs_reg=SLOTS,
                    elem_size=C,
                    transpose=True,
                    single_packet=False,
                )
                sS = ssp.tile([128, NCHK * TILE], bf, tag="sS")
                nc.sync.dma_start(
                    sS[:],
                    seg_dram[:, t * NCHK * TILE:(t + 1) * NCHK * TILE])

                # per-k conv matmuls -> channel-major contributions in PSUM
                csz = [min(512, SLOTS - 512 * i)
                       for i in range((SLOTS + 511) // 512)]
                c_tiles = [cps.tile([128, csz[i]], f32, tag=f"c{i}",
                                    name=f"c{i}")
                           for i in range(len(csz))]
                for kk in range(NK):
                    a = kk * W_RUN
                    b = a + W_RUN if kk < NK - 1 else SLOTS
                    while a < b:
                        ti = a // 512
                        seg_end = min(b, (ti + 1) * 512)
                        nc.tensor.matmul(
                            c_tiles[ti][:, a - ti * 512:seg_end - ti * 512],
                            lhsT=w_sb[:, kk * C:(kk + 1) * C],
                            rhs=g[:, a:seg_end],
                            start=True,
                            stop=True,
                        )
                        a = seg_end

                c_sb = csbp.tile([128, SLOTS], bf, tag="csb")
                for i in range(len(csz)):
                    nc.scalar.activation(
                        c_sb[:, i * 512:i * 512 + csz[i]], c_tiles[i][:],
                        AF.Identity)

                # transpose pair chunks -> pair-major c_pm
                c_pm = cpmp.tile([128, SLOTS], bf, tag="cpm")
                for bch in range((NCHK + 3) // 4):
                    nch_b = min(4, NCHK - 4 * bch)
                    pt = ptp.tile([128, 512], bf, tag="pt")
                    for cc in range(nch_b):
                        nc.tensor.matmul(
                            pt[:, 128 * cc:128 * (cc + 1)],
                            lhsT=c_sb[:, (4 * bch + cc) * 128:
                                      (4 * bch + cc + 1) * 128],
                            rhs=ident[:],
                            is_transpose=True,
                            start=(cc == 0),
                            stop=(cc == nch_b - 1),
                        )
                    nc.vector.tensor_copy(
                        c_pm[:, bch * 512:bch * 512 + 128 * nch_b],
                        pt[:, 0:128 * nch_b])

                # masked shifted slices for the z-offsets (k=12/14)
                nch = cw.shape[0]
                z12 = selp.tile([nch, TILE], bf, tag="z12")
                nc.vector.tensor_tensor(
                    z12[:], cw[:, 0:TILE], mk[0:nch, 0:TILE],
                    op=mybir.AluOpType.mult)
                z14 = selp.tile([nch, TILE], bf, tag="z14")
                nc.vector.tensor_tensor(
                    z14[:], cw[:, 2:TILE + 2], mk[0:nch, TILE:2 * TILE],
                    op=mybir.AluOpType.mult)

                # segment-sum + center + z-offsets into fp32 PSUM
                ops = opsp.tile([128, TILE], f32, tag="o")
                nc.tensor.matmul(ops[:], lhsT=wc_sb, rhs=cw[:, 1:TILE + 1],
                                 start=True, stop=False)
                nc.tensor.matmul(ops[:], lhsT=wz_sb[:, 0:C], rhs=z12[:],
                                 start=False, stop=False)
                nc.tensor.matmul(ops[:], lhsT=wz_sb[:, C:2 * C], rhs=z14[:],
                                 start=False, stop=False)
                # S matrices are host-built and DMA'd (on-device DVE builds
                # stalled ~12us/tile against concurrent gather XBAR traffic)
                for cch in range(NCHK):
                    nc.tensor.matmul(
                        ops[:],
                        lhsT=c_pm[:, cch * 128:(cch + 1) * 128],
                        rhs=sS[:, cch * TILE:(cch + 1) * TILE],
                        start=False,
                        stop=(cch == NCHK - 1),
                    )
                return ops

            def emit_body():
                # ---- phase 1: h1 = silu(conv1(x)) over the h1 window ----
                for t in range(NT1):
                    xc = ctrp.tile([16, TILE + 2], bf, tag="xc")
                    nc.sync.dma_start(
                        xc[:], x_cm[:, t * TILE:t * TILE + TILE + 2])
                    mk1 = segp.tile([128, 2 * TILE], bf, tag="mk")
                    nc.sync.dma_start(
                        mk1[:], m1[:, t * 2 * TILE:(t + 1) * 2 * TILE])
                    ops = conv_tile(x_tab, i1, s1, w1_sb, w1c_sb[:],
                                    w1z_sb[:], xc[:], mk1[:], t)
                    h1c = actp.tile([C, TILE], bf, tag="h")
                    nc.scalar.activation(h1c[:], ops[:], AF.Silu,
                                         bias=b1_sb[:, 0:1])
                    nc.sync.dma_start(
                        h1cm[:, 1 + t * TILE:1 + (t + 1) * TILE], h1c[:])
                    # transpose -> row-major h1 table (swizzled rows)
                    pt = ptp.tile([128, 512], bf, tag="pt")
                    for cc in range(4):
                        nc.tensor.matmul(
                            pt[:, 128 * cc:128 * (cc + 1)],
                            lhsT=h1c[:, 128 * cc:128 * (cc + 1)],
                            rhs=ident[:],
                            is_transpose=True,
                            start=(cc == 0),
                            stop=(cc == 3),
                        )
                    st = stagep.tile([C, TILE], bf, tag="st")
                    nc.vector.tensor_copy(st[:], pt[:])
                    r0 = NZ + t * TILE
                    nc.sync.dma_start(
                        h1tab[r0:r0 + TILE, :].rearrange(
                            "(p c) e -> p (c e)", c=4),
                        st[:],
                    )

                # ---- phase 2: conv2 + pointwise MLP ----
                for t in range(NT2):
                    hc = ctrp.tile([C, TILE + 2], bf, tag="hc")
                    nc.sync.dma_start(
                        hc[:],
                        h1cm[:, D_HALO + t * TILE:
                              D_HALO + t * TILE + TILE + 2])
                    mk2 = segp.tile([128, 2 * TILE], bf, tag="mk")
                    nc.sync.dma_start(
                        mk2[:], m2[:, t * 2 * TILE:(t + 1) * 2 * TILE])
                    ops = conv_tile(h1tab, i2, s2, w2_sb, w2c_sb[:],
                                    w2z_sb[:], hc[:], mk2[:], t)
                    h2 = actp.tile([C, TILE], bf, tag="h")
                    nc.scalar.activation(h2[:], ops[:], AF.Silu,
                                         bias=b2_sb[:, 0:1])
                    ps3 = mpsp.tile([C, TILE], f32, tag="w3t")
                    nc.tensor.matmul(ps3[:], lhsT=w3_sb[:], rhs=h2[:],
                                     start=True, stop=True)
                    h3 = actp.tile([C, TILE], bf, tag="h")
                    nc.scalar.activation(h3[:], ps3[:], AF.Silu,
                                         bias=b3_sb[:, 0:1])
                    ps4 = mpsp.tile([1, TILE], f32, tag="o4t")
                    nc.tensor.matmul(ps4[:], lhsT=w4_sb[:, 0:1], rhs=h3[:],
                                     start=True, stop=True)
                    ot = outp.tile([1, TILE], f32, tag="ot")
                    nc.scalar.activation(
                        ot[0:1, :], ps4[:], AF.Identity,
                        bias=b4_sb[0:1, 0:1])
                    nc.sync.dma_start(
                        outd[None, t * TILE:(t + 1) * TILE], ot[0:1, :])

            if bench_reps > 0:
                with tc.For_i(0, bench_reps, 1):
                    emit_body()
            else:
                emit_body()

    nc.compile()
    return nc


_NC_CACHE = {}


def _get_nc():
    if "nc" not in _NC_CACHE:
        _NC_CACHE["nc"] = _build_program()
    return _NC_CACHE["nc"]


# ------------------------------------------------------------------ host prep
def _sinusoidal(t):
    half = TEMB // 2
    freqs = (np.float32(2.0) ** np.arange(half, dtype=np.float32)) * np.float32(np.pi)
    ang = t.astype(np.float32)[:, None] * freqs[None, :]
    return np.concatenate([np.sin(ang), np.cos(ang)], -1).astype(np.float32)


def _wrap_idx_tiles(I):
    """[T, SLOTS] int -> [128, T*IDXC] int16 (16-wrap, replicated x8)."""
    T = I.shape[0]
    a = I.reshape(T, SLOTS // 16, 16).transpose(2, 0, 1).reshape(
        16, T * (SLOTS // 16))
    return np.tile(a, (8, 1)).astype(np.int16)


def _s_tiles(S):
    """[T, SLOTS] segids -> [128, T*NCHK*TILE] bf16 one-hot S matrices."""
    T = S.shape[0]
    seg = S.reshape(T, NCHK, 128).transpose(2, 0, 1).reshape(128, T * NCHK)
    valid = seg < TILE
    segc = np.clip(seg, 0, TILE - 1).astype(np.int64)
    arr = np.zeros((128, T * NCHK, TILE), _bf16)
    np.put_along_axis(arr, segc[:, :, None], _bf16(1.0), axis=2)
    arr[~valid, :] = 0
    return arr.reshape(128, T * NCHK * TILE)


def _mask_tiles(M):
    """[T, 2, TILE] -> [128, T*2*TILE] bf16 replicated across partitions."""
    T = M.shape[0]
    flat = M.reshape(1, T * 2 * TILE)
    return np.broadcast_to(flat, (128, flat.shape[1])).astype(_bf16)


def _phys_h1_row(j):
    """Logical h1-window row -> physical row in the h1 table."""
    t = j // TILE
    r = j % TILE
    return t * TILE + 4 * (r % 128) + r // 128


def _prep_core(core, x_full, nidx):
    s = core * PER
    e = s + PER
    wlo = s - D_HALO

    sub2 = nidx[:, s:e]
    v2 = sub2[sub2 < N]
    lo1 = int(min(v2.min(), s))
    hi1 = int(max(v2.max() + 1, e))
    assert wlo <= lo1 and hi1 <= wlo + M_H1, (core, wlo, lo1, hi1)

    sub1 = nidx[:, lo1:hi1]
    v1 = sub1[sub1 < N]
    lo0 = int(min(v1.min(), lo1))
    hi0 = int(max(v1.max() + 1, hi1))
    n0 = hi0 - lo0
    assert n0 <= XT - NZ, (core, n0)

    x_tab = np.zeros((XT, C), _bf16)
    x_tab[NZ:NZ + n0, :IN_CH] = x_full[lo0:hi0].astype(_bf16)

    x_cm = np.zeros((16, CMW), _bf16)
    wa = max(0, -wlo)
    wb = min(M_H1, N - wlo)
    x_cm[:IN_CH, 1 + wa:1 + wb] = x_full[wlo + wa:wlo + wb].T.astype(_bf16)

    rng = np.random.default_rng(12345 + core)

    def build_streams(nt, out_base, olo, ohi, src_map):
        I = rng.integers(0, NZ, size=(nt, SLOTS)).astype(np.int32)
        S = np.full((nt, SLOTS), SEG_DUMP, np.float32)
        M = np.zeros((nt, 2, TILE), np.float32)
        for t in range(nt):
            a = out_base + t * TILE
            rows = np.arange(a, a + TILE)
            m = (rows >= olo) & (rows < ohi)
            if not m.any():
                continue
            blk = nidx[:, np.clip(rows, 0, N - 1)]
            for kk, k in enumerate(KS):
                j = np.where(m, blk[k], N)
                valid = j < N
                ii = np.nonzero(valid)[0]
                nv = len(ii)
                assert nv <= W_RUN, (core, t, k, nv)
                base = kk * W_RUN
                I[t, base:base + nv] = src_map(j[valid])
                S[t, base:base + nv] = ii
            M[t, 0] = m & (blk[12] < N)
            M[t, 1] = m & (blk[14] < N)
        return I, S, M

    # conv1: outputs = h1 window rows (valid range [lo1, hi1)), sources = x
    I1, S1, M1 = build_streams(
        NT1, wlo, lo1, hi1, lambda j: NZ + (j.astype(np.int64) - lo0))
    # conv2: outputs = [s, e), sources = h1 table (swizzled rows)
    I2, S2, M2 = build_streams(
        NT2, s, s, e,
        lambda j: NZ + _phys_h1_row(j.astype(np.int64) - wlo))

    assert I1.max() < 32768 and I2.max() < 32768
    return {
        "x_tab": x_tab,
        "x_cm": x_cm,
        "i1": _wrap_idx_tiles(I1),
        "s1": _s_tiles(S1),
        "i2": _wrap_idx_tiles(I2),
        "s2": _s_tiles(S2),
        "m1": _mask_tiles(M1),
        "m2": _mask_tiles(M2),
    }


def _prep_shared(W1, b1, W2, b2, W3, b3, W4, b4):
    w1d = np.zeros((C, NK * C), np.float32)
    for kk, k in enumerate(KS):
        w1d[:IN_CH, kk * C:(kk + 1) * C] = W1[k]
    w1cd = np.zeros((16, C), np.float32)
    w1cd[:IN_CH] = W1[13]
    w1zd = np.zeros((16, 2 * C), np.float32)
    w1zd[:IN_CH, 0:C] = W1[12]
    w1zd[:IN_CH, C:2 * C] = W1[14]
    w2d = np.zeros((C, NK * C), np.float32)
    for kk, k in enumerate(KS):
        w2d[:, kk * C:(kk + 1) * C] = W2[k]
    w2zd = np.concatenate([W2[12], W2[14]], axis=1)
    w4d = np.zeros((C, 16), _bf16)
    w4d[:, 0] = W4[:, 0].astype(_bf16)
    return {
        "w1": w1d.astype(_bf16),
        "w1c": w1cd.astype(_bf16),
        "w2": w2d.astype(_bf16),
        "w1z": w1zd.astype(_bf16),
        "w2c": np.ascontiguousarray(W2[13]).astype(_bf16),
        "w2z": np.ascontiguousarray(w2zd).astype(_bf16),
        "w3": np.ascontiguousarray(W3).astype(_bf16),
        "w4": w4d,
        "b1": np.ascontiguousarray(b1.reshape(C, 1), dtype=np.float32),
        "b2": np.ascontiguousarray(b2.reshape(C, 1), dtype=np.float32),
        "b3": np.ascontiguousarray(b3.reshape(C, 1), dtype=np.float32),
        "b4": np.ascontiguousarray(b4.reshape(1, 1), dtype=np.float32),
    }


def _run_pjrt(nc, in_maps, reps=0):
    """Execute the Bass program on the 8 axon-tunneled cores via PJRT."""
    import time as _time
    import jax
    from jax.sharding import Mesh, NamedSharding, PartitionSpec
    from jax.experimental.shard_map import shard_map
    import concourse.mybir as mybir
    from concourse import bass2jax

    bass2jax.install_neuronx_cc_hook()

    n_cores = len(in_maps)
    partition_name = (
        nc.partition_id_tensor.name if nc.partition_id_tensor else None
    )
    in_names, out_names, out_avals, zero_outs = [], [], [], []
    for alloc in nc.m.functions[0].allocations:
        if not isinstance(alloc, mybir.MemoryLocationSet):
            continue
        name = alloc.memorylocations[0].name
        if alloc.kind == "ExternalInput":
            if name != partition_name:
                in_names.append(name)
        elif alloc.kind == "ExternalOutput":
            shape = tuple(alloc.tensor_shape)
            dtype = mybir.dt.np(alloc.dtype)
            out_names.append(name)
            out_avals.append(jax.core.ShapedArray(shape, dtype))
            zero_outs.append(np.zeros(shape, dtype))
    n_params = len(in_names)
    n_outs = len(out_names)
    all_names = in_names + out_names
    if partition_name is not None:
        all_names = all_names + [partition_name]
    donate = tuple(range(n_params, n_params + n_outs))

    def _body(*args):
        operands = list(args)
        if partition_name is not None:
            operands.append(bass2jax.partition_id_tensor())
        outs = bass2jax._bass_exec_p.bind(
            *operands,
            out_avals=tuple(out_avals),
            in_names=tuple(all_names),
            out_names=tuple(out_names),
            lowering_input_output_aliases=(),
            sim_require_finite=True,
            sim_require_nnan=True,
            nc=nc,
        )
        return tuple(outs)

    devices = jax.devices()[:n_cores]
    mesh = Mesh(np.asarray(devices), ("core",))
    spec = PartitionSpec("core")
    sharded = jax.jit(
        shard_map(_body, mesh=mesh, in_specs=(spec,) * (n_params + n_outs),
                  out_specs=(spec,) * n_outs, check_rep=False),
        donate_argnums=donate,
        keep_unused=True,
    )
    concat_in = [
        np.concatenate([np.asarray(m[name]) for m in in_maps], axis=0)
        for name in in_names
    ]
    sh = NamedSharding(mesh, spec)
    inp_dev = [jax.device_put(a, sh) for a in concat_in]

    def _zeros():
        return [np.zeros((n_cores * z.shape[0], *z.shape[1:]), z.dtype)
                for z in zero_outs]

    out_arrs = sharded(*inp_dev, *_zeros())
    jax.block_until_ready(out_arrs)
    results = [
        {name: np.asarray(out_arrs[i]).reshape(n_cores, *out_avals[i].shape)[c]
         for i, name in enumerate(out_names)}
        for c in range(n_cores)
    ]

    times = []
    for _ in range(reps):
        zs = _zeros()
        t0 = _time.perf_counter()
        o = sharded(*inp_dev, *zs)
        jax.block_until_ready(o)
        times.append(_time.perf_counter() - t0)
    return results, times


def _prep_in_maps(inputs):
    features = np.asarray(inputs["features"], np.float32)
    t = np.asarray(inputs["t"])
    nidx = np.asarray(inputs["neighbor_idx"]).astype(np.int32)
    x_full = np.concatenate([features, _sinusoidal(t)], -1)
    shared = _prep_shared(
        np.asarray(inputs["W1"], np.float32), np.asarray(inputs["b1"], np.float32),
        np.asarray(inputs["W2"], np.float32), np.asarray(inputs["b2"], np.float32),
        np.asarray(inputs["W3"], np.float32), np.asarray(inputs["b3"], np.float32),
        np.asarray(inputs["W4"], np.float32), np.asarray(inputs["b4"], np.float32),
    )
    in_maps = []
    for core in range(NCORES):
        m = _prep_core(core, x_full, nidx)
        m.update(shared)
        in_maps.append(m)
    return in_maps


def _run(inputs, reps=0):
    in_maps = _prep_in_maps(inputs)
    nc = _get_nc()
    results, times = _run_pjrt(nc, in_maps, reps=reps)
    out = np.empty((N, 1), np.float32)
    for core in range(NCORES):
        out[core * PER:(core + 1) * PER, 0] = results[core]["out"][:PER]
    return out, times


def kernel(**inputs) -> np.ndarray:
    out, _ = _run(inputs, reps=0)
    return out


def bench(inputs, loop_reps=(1, 26), wall_reps=8):
    """Estimate on-device kernel time by diffing wall times of programs that
    loop the whole body R1 vs R2 times on-device."""
    in_maps = _prep_in_maps(inputs)
    walls = {}
    outs = {}
    for R in loop_reps:
        nc = _build_program(bench_reps=R)
        results, times = _run_pjrt(nc, in_maps, reps=wall_reps)
        walls[R] = min(times)
        out = np.empty((N, 1), np.float32)
        for core in range(NCORES):
            out[core * PER:(core + 1) * PER, 0] = results[core]["out"][:PER]
        outs[R] = out
    R1, R2 = loop_reps
    per_iter = (walls[R2] - walls[R1]) / (R2 - R1)
    return per_iter, walls, outs


# revision 14
# speedup vs baseline: 2.9173x; 2.9173x over previous
"""Trainium2 Bass kernel for nn_DiffusionCNN (submanifold sparse 3x3x3 CNN).

Valid-pair design (8-core SPMD, no collectives):
  - The 27-offset rulebook is ~91% empty (uniform 9.5% occupancy); the
    baseline gathered every slot (1.45M rows/core) and was bound by SWDGE
    descriptor generation on the GpSimd engine (~7.8ns/idx, 11.6ms busy).
  - This kernel gathers only valid neighbor pairs, grouped per output tile
    (512 rows) and per non-center offset k as fixed-width runs (W_RUN=78
    covers the global max run of 73).  Per tile: 26*78 -> 2048 gather slots
    (incl. pads to zero rows), i.e. ~7x fewer descriptors.
  - Per k-run a single matmul (lhsT=W_k) produces channel-major
    contributions in PSUM; PE transposes flip them pair-major; the
    pair->output routing runs as a segment-sum matmul against 0/1 selection
    matrices built on the DVE (iota fp16 is_equal per-partition segids).
    Accumulation stays in fp32 PSUM.  The center offset (k=13, identity)
    is applied densely (one matmul from a channel-major x/h1 slice).
  - h1 window is fixed at [s-2048, s-2048+29184) for every core so the
    shared program uses static offsets; per-core variation lives entirely
    in the index/segid streams.
  - All matmuls bf16 with fp32 PSUM accumulation.

Host-side work: slicing inputs, building per-core gather index + segid
streams (int16/fp16), packing weights, re-assembling the output.
"""

import numpy as np
import ml_dtypes

# ---------------------------------------------------------------- constants
N = 200000
PER = 25000
NCORES = 8
C = 128
K = 27
TEMB = 6
IN_CH = 7  # features(1) + sin/cos(6)

TILE = 512
D_HALO = 2048            # fixed h1-window offset (512-aligned, >= max halo 1619)
NT1 = 57                 # h1 window tiles per core
NT2 = 49                 # output tiles per core
M_H1 = NT1 * TILE        # 29184 h1 rows computed per core
M_OUT = NT2 * TILE       # 25088 output rows per core (25000 + pad)
NZ = 1024                # zero rows at the front of each gather table
XT = 32384               # x table rows (NZ + up to 31350 real, max idx 32373)
H1T = NZ + M_H1          # 30208 h1 table rows (max idx 30207)

# k=13 is the center (identity); k=12/14 are (0,0,-1)/(0,0,+1) whose valid
# neighbors are always the adjacent table rows (z-sorted order) -> handled
# densely via shifted channel-major slices times a 0/1 mask.
KS = [k for k in range(K) if k not in (12, 13, 14)]
NK = 24
W_RUN = 73               # run slots per (tile, k); global max run is 73
SLOTS = 1792             # 24*73=1752 run slots + 40 tail pads; 14 chunks of 128
NCHK = SLOTS // 128      # 14
IDXC = SLOTS // 16       # idx columns per tile (112)
CMW = M_H1 + 2           # ch-major tables get a zero border column each side

_bf16 = ml_dtypes.bfloat16
_f16 = np.float16
SEG_DUMP = 999.0         # segid for pad slots; iota is 0..511 so never equal


# ------------------------------------------------------------- device program
def _build_program(bench_reps=0):
    import concourse.bass as bass
    import concourse.mybir as mybir
    import concourse.tile as tile
    from concourse import bacc
    from concourse.masks import make_identity

    bf = mybir.dt.bfloat16
    f32 = mybir.dt.float32
    f16 = mybir.dt.float16
    i16 = mybir.dt.int16
    AF = mybir.ActivationFunctionType

    nc = bacc.Bacc("TRN2", target_bir_lowering=False, debug=False)

    x_tab = nc.dram_tensor("x_tab", [XT, C], bf, kind="ExternalInput")
    x_cm = nc.dram_tensor("x_cm", [16, CMW], bf, kind="ExternalInput")
    i1 = nc.dram_tensor("i1", [128, NT1 * IDXC], i16, kind="ExternalInput")
    s1 = nc.dram_tensor("s1", [128, NT1 * NCHK * TILE], bf, kind="ExternalInput")
    i2 = nc.dram_tensor("i2", [128, NT2 * IDXC], i16, kind="ExternalInput")
    s2 = nc.dram_tensor("s2", [128, NT2 * NCHK * TILE], bf, kind="ExternalInput")
    w1 = nc.dram_tensor("w1", [C, NK * C], bf, kind="ExternalInput")
    w1c = nc.dram_tensor("w1c", [16, C], bf, kind="ExternalInput")
    w1z = nc.dram_tensor("w1z", [16, 2 * C], bf, kind="ExternalInput")
    w2 = nc.dram_tensor("w2", [C, NK * C], bf, kind="ExternalInput")
    w2c = nc.dram_tensor("w2c", [C, C], bf, kind="ExternalInput")
    w2z = nc.dram_tensor("w2z", [C, 2 * C], bf, kind="ExternalInput")
    m1 = nc.dram_tensor("m1", [128, NT1 * 2 * TILE], bf, kind="ExternalInput")
    m2 = nc.dram_tensor("m2", [128, NT2 * 2 * TILE], bf, kind="ExternalInput")
    w3 = nc.dram_tensor("w3", [C, C], bf, kind="ExternalInput")
    w4 = nc.dram_tensor("w4", [C, 16], bf, kind="ExternalInput")
    b1 = nc.dram_tensor("b1", [C, 1], f32, kind="ExternalInput")
    b2 = nc.dram_tensor("b2", [C, 1], f32, kind="ExternalInput")
    b3 = nc.dram_tensor("b3", [C, 1], f32, kind="ExternalInput")
    b4 = nc.dram_tensor("b4", [1, 1], f32, kind="ExternalInput")
    outd = nc.dram_tensor("out", [M_OUT], f32, kind="ExternalOutput")
    h1tab = nc.dram_tensor("h1_tab", [H1T, C], bf, kind="Internal")
    h1cm = nc.dram_tensor("h1_cm", [128, CMW], bf, kind="Internal")

    with tile.TileContext(nc) as tc:
        with (
            tc.tile_pool(name="const", bufs=1) as constp,
            tc.tile_pool(name="idx", bufs=3) as idxp,
            tc.tile_pool(name="seg", bufs=3) as segp,
            tc.tile_pool(name="ctr", bufs=3) as ctrp,
            tc.tile_pool(name="gat", bufs=3) as gatp,
            tc.tile_pool(name="csb", bufs=2) as csbp,
            tc.tile_pool(name="cpm", bufs=2) as cpmp,
            tc.tile_pool(name="sel", bufs=3) as selp,
            tc.tile_pool(name="sS", bufs=2) as ssp,
            tc.tile_pool(name="act", bufs=3) as actp,
            tc.tile_pool(name="stage", bufs=2) as stagep,
            tc.tile_pool(name="osb", bufs=2) as outp,
            tc.tile_pool(name="cps", bufs=1, space="PSUM") as cps,
            tc.tile_pool(name="ptp", bufs=1, space="PSUM") as ptp,
            tc.tile_pool(name="ops", bufs=1, space="PSUM") as opsp,
            tc.tile_pool(name="mps", bufs=1, space="PSUM") as mpsp,
        ):
            w1_sb = constp.tile([C, NK * C], bf, tag="w1")
            nc.sync.dma_start(w1_sb[:], w1[:])
            w1c_sb = constp.tile([16, C], bf, tag="w1c")
            nc.sync.dma_start(w1c_sb[:], w1c[:])
            w1z_sb = constp.tile([16, 2 * C], bf, tag="w1z")
            nc.sync.dma_start(w1z_sb[:], w1z[:])
            w2_sb = constp.tile([C, NK * C], bf, tag="w2")
            nc.sync.dma_start(w2_sb[:], w2[:])
            w2c_sb = constp.tile([C, C], bf, tag="w2c")
            nc.sync.dma_start(w2c_sb[:], w2c[:])
            w2z_sb = constp.tile([C, 2 * C], bf, tag="w2z")
            nc.sync.dma_start(w2z_sb[:], w2z[:])
            w3_sb = constp.tile([C, C], bf, tag="w3")
            nc.sync.dma_start(w3_sb[:], w3[:])
            w4_sb = constp.tile([C, 16], bf, tag="w4")
            nc.sync.dma_start(w4_sb[:], w4[:])
            b1_sb = constp.tile([C, 1], f32, tag="b1")
            nc.sync.dma_start(b1_sb[:], b1[:])
            b2_sb = constp.tile([C, 1], f32, tag="b2")
            nc.sync.dma_start(b2_sb[:], b2[:])
            b3_sb = constp.tile([C, 1], f32, tag="b3")
            nc.sync.dma_start(b3_sb[:], b3[:])
            b4_sb = constp.tile([1, 1], f32, tag="b4")
            nc.sync.dma_start(b4_sb[:], b4[:])
            ident = constp.tile([C, C], bf, tag="ident")
            make_identity(nc, ident[:])
            zblk = constp.tile([C, NZ // 128 * C], bf, tag="zblk")
            nc.vector.memset(zblk[:], 0.0)
            nc.sync.dma_start(
                h1tab[0:NZ, :].rearrange("(p c) e -> p (c e)", c=NZ // 128),
                zblk[:])

            ztiny = constp.tile([128, 1], bf, tag="ztiny")
            nc.vector.memset(ztiny[:], 0.0)
            nc.sync.dma_start(h1cm[:, 0:1], ztiny[:])
            nc.sync.dma_start(h1cm[:, CMW - 1:CMW], ztiny[:])


            def conv_tile(src_tab, idx_dram, seg_dram, w_sb, wc_sb, wz_sb,
                          cw, mk, t):
                it = idxp.tile([128, IDXC], mybir.dt.int16, tag="it")
                nc.sync.dma_start(
                    it[:], idx_dram[:, t * IDXC:(t + 1) * IDXC])
                g = gatp.tile([128, SLOTS], bf, tag="g")
                nc.gpsimd.dma_gather(
                    out_ap=g[:].rearrange("p (o n) -> p o n", o=1),
                    in_ap=src_tab[:, :],
                    idxs_ap=it[:, :],
                    num_idxs=SLOTS,
                    num_idxs_reg=SLOTS,
                    elem_size=C,
                    transpose=True,
                    single_packet=False,
                )
                sS = ssp.tile([128, NCHK * TILE], bf, tag="sS")
                nc.sync.dma_start(
                    sS[:],
                    seg_dram[:, t * NCHK * TILE:(t + 1) * NCHK * TILE])

                # per-k conv matmuls -> channel-major contributions in PSUM
                csz = [min(512, SLOTS - 512 * i)
                       for i in range((SLOTS + 511) // 512)]
                c_tiles = [cps.tile([128, csz[i]], f32, tag=f"c{i}",
                                    name=f"c{i}")
                           for i in range(len(csz))]
                for kk in range(NK):
                    a = kk * W_RUN
                    b = a + W_RUN if kk < NK - 1 else SLOTS
                    while a < b:
                        ti = a // 512
                        seg_end = min(b, (ti + 1) * 512)
                        nc.tensor.matmul(
                            c_tiles[ti][:, a - ti * 512:seg_end - ti * 512],
                            lhsT=w_sb[:, kk * C:(kk + 1) * C],
                            rhs=g[:, a:seg_end],
                            start=True,
                            stop=True,
                        )
                        a = seg_end

                c_sb = csbp.tile([128, SLOTS], bf, tag="csb")
                for i in range(len(csz)):
                    nc.scalar.activation(
                        c_sb[:, i * 512:i * 512 + csz[i]], c_tiles[i][:],
                        AF.Identity)

                # transpose pair chunks -> pair-major c_pm
                c_pm = cpmp.tile([128, SLOTS], bf, tag="cpm")
                for bch in range((NCHK + 3) // 4):
                    nch_b = min(4, NCHK - 4 * bch)
                    pt = ptp.tile([128, 512], bf, tag="pt")
                    for cc in range(nch_b):
                        nc.tensor.matmul(
                            pt[:, 128 * cc:128 * (cc + 1)],
                            lhsT=c_sb[:, (4 * bch + cc) * 128:
                                      (4 * bch + cc + 1) * 128],
                            rhs=ident[:],
                            is_transpose=True,
                            start=(cc == 0),
                            stop=(cc == nch_b - 1),
                        )
                    nc.vector.tensor_copy(
                        c_pm[:, bch * 512:bch * 512 + 128 * nch_b],
                        pt[:, 0:128 * nch_b])

                # masked shifted slices for the z-offsets (k=12/14)
                nch = cw.shape[0]
                z12 = selp.tile([nch, TILE], bf, tag="z12")
                nc.vector.tensor_tensor(
                    z12[:], cw[:, 0:TILE], mk[0:nch, 0:TILE],
                    op=mybir.AluOpType.mult)
                z14 = selp.tile([nch, TILE], bf, tag="z14")
                nc.vector.tensor_tensor(
                    z14[:], cw[:, 2:TILE + 2], mk[0:nch, TILE:2 * TILE],
                    op=mybir.AluOpType.mult)

                # segment-sum + center + z-offsets into fp32 PSUM
                ops = opsp.tile([128, TILE], f32, tag="o")
                nc.tensor.matmul(ops[:], lhsT=wc_sb, rhs=cw[:, 1:TILE + 1],
                                 start=True, stop=False)
                nc.tensor.matmul(ops[:], lhsT=wz_sb[:, 0:C], rhs=z12[:],
                                 start=False, stop=False)
                nc.tensor.matmul(ops[:], lhsT=wz_sb[:, C:2 * C], rhs=z14[:],
                                 start=False, stop=False)
                # S matrices are host-built and DMA'd (on-device DVE builds
                # stalled ~12us/tile against concurrent gather XBAR traffic)
                for cch in range(NCHK):
                    nc.tensor.matmul(
                        ops[:],
                        lhsT=c_pm[:, cch * 128:(cch + 1) * 128],
                        rhs=sS[:, cch * TILE:(cch + 1) * TILE],
                        start=False,
                        stop=(cch == NCHK - 1),
                    )
                return ops

            def emit_body():
                # ---- phase 1: h1 = silu(conv1(x)) over the h1 window ----
                for t in range(NT1):
                    xc = ctrp.tile([16, TILE + 2], bf, tag="xc")
                    nc.sync.dma_start(
                        xc[:], x_cm[:, t * TILE:t * TILE + TILE + 2])
                    mk1 = segp.tile([128, 2 * TILE], bf, tag="mk")
                    nc.sync.dma_start(
                        mk1[:], m1[:, t * 2 * TILE:(t + 1) * 2 * TILE])
                    ops = conv_tile(x_tab, i1, s1, w1_sb, w1c_sb[:],
                                    w1z_sb[:], xc[:], mk1[:], t)
                    h1c = actp.tile([C, TILE], bf, tag="h")
                    nc.scalar.activation(h1c[:], ops[:], AF.Silu,
                                         bias=b1_sb[:, 0:1])
                    nc.sync.dma_start(
                        h1cm[:, 1 + t * TILE:1 + (t + 1) * TILE], h1c[:])
                    # transpose -> row-major h1 table (swizzled rows)
                    pt = ptp.tile([128, 512], bf, tag="pt")
                    for cc in range(4):
                        nc.tensor.matmul(
                            pt[:, 128 * cc:128 * (cc + 1)],
                            lhsT=h1c[:, 128 * cc:128 * (cc + 1)],
                            rhs=ident[:],
                            is_transpose=True,
                            start=(cc == 0),
                            stop=(cc == 3),
                        )
                    st = stagep.tile([C, TILE], bf, tag="st")
                    nc.vector.tensor_copy(st[:], pt[:])
                    r0 = NZ + t * TILE
                    nc.sync.dma_start(
                        h1tab[r0:r0 + TILE, :].rearrange(
                            "(p c) e -> p (c e)", c=4),
                        st[:],
                    )

                # ---- phase 2: conv2 + pointwise MLP ----
                for t in range(NT2):
                    hc = ctrp.tile([C, TILE + 2], bf, tag="hc")
                    nc.sync.dma_start(
                        hc[:],
                        h1cm[:, D_HALO + t * TILE:
                              D_HALO + t * TILE + TILE + 2])
                    mk2 = segp.tile([128, 2 * TILE], bf, tag="mk")
                    nc.sync.dma_start(
                        mk2[:], m2[:, t * 2 * TILE:(t + 1) * 2 * TILE])
                    ops = conv_tile(h1tab, i2, s2, w2_sb, w2c_sb[:],
                                    w2z_sb[:], hc[:], mk2[:], t)
                    h2 = actp.tile([C, TILE], bf, tag="h")
                    nc.scalar.activation(h2[:], ops[:], AF.Silu,
                                         bias=b2_sb[:, 0:1])
                    ps3 = mpsp.tile([C, TILE], f32, tag="w3t")
                    nc.tensor.matmul(ps3[:], lhsT=w3_sb[:], rhs=h2[:],
                                     start=True, stop=True)
                    h3 = actp.tile([C, TILE], bf, tag="h")
                    nc.scalar.activation(h3[:], ps3[:], AF.Silu,
                                         bias=b3_sb[:, 0:1])
                    ps4 = mpsp.tile([1, TILE], f32, tag="o4t")
                    nc.tensor.matmul(ps4[:], lhsT=w4_sb[:, 0:1], rhs=h3[:],
                                     start=True, stop=True)
                    ot = outp.tile([1, TILE], f32, tag="ot")
                    nc.scalar.activation(
                        ot[0:1, :], ps4[:], AF.Identity,
                        bias=b4_sb[0:1, 0:1])
                    nc.sync.dma_start(
                        outd[None, t * TILE:(t + 1) * TILE], ot[0:1, :])

            if bench_reps > 0:
                with tc.For_i(0, bench_reps, 1):
                    emit_body()
            else:
                emit_body()

    nc.compile()
    return nc


_NC_CACHE = {}


def _get_nc():
    if "nc" not in _NC_CACHE:
        _NC_CACHE["nc"] = _build_program()
    return _NC_CACHE["nc"]


# ------------------------------------------------------------------ host prep
def _sinusoidal(t):
    half = TEMB // 2
    freqs = (np.float32(2.0) ** np.arange(half, dtype=np.float32)) * np.float32(np.pi)
    ang = t.astype(np.float32)[:, None] * freqs[None, :]
    return np.concatenate([np.sin(ang), np.cos(ang)], -1).astype(np.float32)


def _wrap_idx_tiles(I):
    """[T, SLOTS] int -> [128, T*IDXC] int16 (16-wrap, replicated x8)."""
    T = I.shape[0]
    a = I.reshape(T, SLOTS // 16, 16).transpose(2, 0, 1).reshape(
        16, T * (SLOTS // 16))
    return np.tile(a, (8, 1)).astype(np.int16)


def _s_tiles(S):
    """[T, SLOTS] segids -> [128, T*NCHK*TILE] bf16 one-hot S matrices."""
    T = S.shape[0]
    seg = S.reshape(T, NCHK, 128).transpose(2, 0, 1).reshape(128, T * NCHK)
    valid = seg < TILE
    segc = np.clip(seg, 0, TILE - 1).astype(np.int64)
    arr = np.zeros((128, T * NCHK, TILE), _bf16)
    np.put_along_axis(arr, segc[:, :, None], _bf16(1.0), axis=2)
    arr[~valid, :] = 0
    return arr.reshape(128, T * NCHK * TILE)


def _mask_tiles(M):
    """[T, 2, TILE] -> [128, T*2*TILE] bf16 replicated across partitions."""
    T = M.shape[0]
    flat = M.reshape(1, T * 2 * TILE)
    return np.broadcast_to(flat, (128, flat.shape[1])).astype(_bf16)


def _phys_h1_row(j):
    """Logical h1-window row -> physical row in the h1 table."""
    t = j // TILE
    r = j % TILE
    return t * TILE + 4 * (r % 128) + r // 128


def _prep_core(core, x_full, nidx):
    s = core * PER
    e = s + PER
    wlo = s - D_HALO

    sub2 = nidx[:, s:e]
    v2 = sub2[sub2 < N]
    lo1 = int(min(v2.min(), s))
    hi1 = int(max(v2.max() + 1, e))
    assert wlo <= lo1 and hi1 <= wlo + M_H1, (core, wlo, lo1, hi1)

    sub1 = nidx[:, lo1:hi1]
    v1 = sub1[sub1 < N]
    lo0 = int(min(v1.min(), lo1))
    hi0 = int(max(v1.max() + 1, hi1))
    n0 = hi0 - lo0
    assert n0 <= XT - NZ, (core, n0)

    x_tab = np.zeros((XT, C), _bf16)
    x_tab[NZ:NZ + n0, :IN_CH] = x_full[lo0:hi0].astype(_bf16)

    x_cm = np.zeros((16, CMW), _bf16)
    wa = max(0, -wlo)
    wb = min(M_H1, N - wlo)
    x_cm[:IN_CH, 1 + wa:1 + wb] = x_full[wlo + wa:wlo + wb].T.astype(_bf16)

    rng = np.random.default_rng(12345 + core)

    def build_streams(nt, out_base, olo, ohi, src_map):
        I = rng.integers(0, NZ, size=(nt, SLOTS)).astype(np.int32)
        S = np.full((nt, SLOTS), SEG_DUMP, np.float32)
        M = np.zeros((nt, 2, TILE), np.float32)
        for t in range(nt):
            a = out_base + t * TILE
            rows = np.arange(a, a + TILE)
            m = (rows >= olo) & (rows < ohi)
            if not m.any():
                continue
            blk = nidx[:, np.clip(rows, 0, N - 1)]
            for kk, k in enumerate(KS):
                j = np.where(m, blk[k], N)
                valid = j < N
                ii = np.nonzero(valid)[0]
                nv = len(ii)
                assert nv <= W_RUN, (core, t, k, nv)
                base = kk * W_RUN
                I[t, base:base + nv] = src_map(j[valid])
                S[t, base:base + nv] = ii
            M[t, 0] = m & (blk[12] < N)
            M[t, 1] = m & (blk[14] < N)
        return I, S, M

    # conv1: outputs = h1 window rows (valid range [lo1, hi1)), sources = x
    I1, S1, M1 = build_streams(
        NT1, wlo, lo1, hi1, lambda j: NZ + (j.astype(np.int64) - lo0))
    # conv2: outputs = [s, e), sources = h1 table (swizzled rows)
    I2, S2, M2 = build_streams(
        NT2, s, s, e,
        lambda j: NZ + _phys_h1_row(j.astype(np.int64) - wlo))

    assert I1.max() < 32768 and I2.max() < 32768
    return {
        "x_tab": x_tab,
        "x_cm": x_cm,
        "i1": _wrap_idx_tiles(I1),
        "s1": _s_tiles(S1),
        "i2": _wrap_idx_tiles(I2),
        "s2": _s_tiles(S2),
        "m1": _mask_tiles(M1),
        "m2": _mask_tiles(M2),
    }


def _prep_shared(W1, b1, W2, b2, W3, b3, W4, b4):
    w1d = np.zeros((C, NK * C), np.float32)
    for kk, k in enumerate(KS):
        w1d[:IN_CH, kk * C:(kk + 1) * C] = W1[k]
    w1cd = np.zeros((16, C), np.float32)
    w1cd[:IN_CH] = W1[13]
    w1zd = np.zeros((16, 2 * C), np.float32)
    w1zd[:IN_CH, 0:C] = W1[12]
    w1zd[:IN_CH, C:2 * C] = W1[14]
    w2d = np.zeros((C, NK * C), np.float32)
    for kk, k in enumerate(KS):
        w2d[:, kk * C:(kk + 1) * C] = W2[k]
    w2zd = np.concatenate([W2[12], W2[14]], axis=1)
    w4d = np.zeros((C, 16), _bf16)
    w4d[:, 0] = W4[:, 0].astype(_bf16)
    return {
        "w1": w1d.astype(_bf16),
        "w1c": w1cd.astype(_bf16),
        "w2": w2d.astype(_bf16),
        "w1z": w1zd.astype(_bf16),
        "w2c": np.ascontiguousarray(W2[13]).astype(_bf16),
        "w2z": np.ascontiguousarray(w2zd).astype(_bf16),
        "w3": np.ascontiguousarray(W3).astype(_bf16),
        "w4": w4d,
        "b1": np.ascontiguousarray(b1.reshape(C, 1), dtype=np.float32),
        "b2": np.ascontiguousarray(b2.reshape(C, 1), dtype=np.float32),
        "b3": np.ascontiguousarray(b3.reshape(C, 1), dtype=np.float32),
        "b4": np.ascontiguousarray(b4.reshape(1, 1), dtype=np.float32),
    }


def _run_pjrt(nc, in_maps, reps=0):
    """Execute the Bass program on the 8 axon-tunneled cores via PJRT."""
    import time as _time
    import jax
    from jax.sharding import Mesh, NamedSharding, PartitionSpec
    from jax.experimental.shard_map import shard_map
    import concourse.mybir as mybir
    from concourse import bass2jax

    bass2jax.install_neuronx_cc_hook()

    n_cores = len(in_maps)
    partition_name = (
        nc.partition_id_tensor.name if nc.partition_id_tensor else None
    )
    in_names, out_names, out_avals, zero_outs = [], [], [], []
    for alloc in nc.m.functions[0].allocations:
        if not isinstance(alloc, mybir.MemoryLocationSet):
            continue
        name = alloc.memorylocations[0].name
        if alloc.kind == "ExternalInput":
            if name != partition_name:
                in_names.append(name)
        elif alloc.kind == "ExternalOutput":
            shape = tuple(alloc.tensor_shape)
            dtype = mybir.dt.np(alloc.dtype)
            out_names.append(name)
            out_avals.append(jax.core.ShapedArray(shape, dtype))
            zero_outs.append(np.zeros(shape, dtype))
    n_params = len(in_names)
    n_outs = len(out_names)
    all_names = in_names + out_names
    if partition_name is not None:
        all_names = all_names + [partition_name]
    donate = tuple(range(n_params, n_params + n_outs))

    def _body(*args):
        operands = list(args)
        if partition_name is not None:
            operands.append(bass2jax.partition_id_tensor())
        outs = bass2jax._bass_exec_p.bind(
            *operands,
            out_avals=tuple(out_avals),
            in_names=tuple(all_names),
            out_names=tuple(out_names),
            lowering_input_output_aliases=(),
            sim_require_finite=True,
            sim_require_nnan=True,
            nc=nc,
        )
        return tuple(outs)

    devices = jax.devices()[:n_cores]
    mesh = Mesh(np.asarray(devices), ("core",))
    spec = PartitionSpec("core")
    sharded = jax.jit(
        shard_map(_body, mesh=mesh, in_specs=(spec,) * (n_params + n_outs),
                  out_specs=(spec,) * n_outs, check_rep=False),
        donate_argnums=donate,
        keep_unused=True,
    )
    concat_in = [
        np.concatenate([np.asarray(m[name]) for m in in_maps], axis=0)
        for name in in_names
    ]
    sh = NamedSharding(mesh, spec)
    inp_dev = [jax.device_put(a, sh) for a in concat_in]

    def _zeros():
        return [np.zeros((n_cores * z.shape[0], *z.shape[1:]), z.dtype)
                for z in zero_outs]

    out_arrs = sharded(*inp_dev, *_zeros())
    jax.block_until_ready(out_arrs)
    results = [
        {name: np.asarray(out_arrs[i]).reshape(n_cores, *out_avals[i].shape)[c]
         for i, name in enumerate(out_names)}
        for c in range(n_cores)
    ]

    times = []
    for _ in range(reps):
        zs = _zeros()
        t0 = _time.perf_counter()
        o = sharded(*inp_dev, *zs)
        jax.block_until_ready(o)
        times.append(_time.perf_counter() - t0)
    return results, times


def _prep_in_maps(inputs):
    features = np.asarray(inputs["features"], np.float32)
    t = np.asarray(inputs["t"])
    nidx = np.asarray(inputs["neighbor_idx"]).astype(np.int32)
    x_full = np.concatenate([features, _sinusoidal(t)], -1)
    shared = _prep_shared(
        np.asarray(inputs["W1"], np.float32), np.asarray(inputs["b1"], np.float32),
        np.asarray(inputs["W2"], np.float32), np.asarray(inputs["b2"], np.float32),
        np.asarray(inputs["W3"], np.float32), np.asarray(inputs["b3"], np.float32),
        np.asarray(inputs["W4"], np.float32), np.asarray(inputs["b4"], np.float32),
    )
    in_maps = []
    for core in range(NCORES):
        m = _prep_core(core, x_full, nidx)
        m.update(shared)
        in_maps.append(m)
    return in_maps


def _run(inputs, reps=0):
    in_maps = _prep_in_maps(inputs)
    nc = _get_nc()
    results, times = _run_pjrt(nc, in_maps, reps=reps)
    out = np.empty((N, 1), np.float32)
    for core in range(NCORES):
        out[core * PER:(core + 1) * PER, 0] = results[core]["out"][:PER]
    return out, times


def kernel(**inputs) -> np.ndarray:
    out, _ = _run(inputs, reps=0)
    return out


def bench(inputs, loop_reps=(1, 26), wall_reps=8):
    """Estimate on-device kernel time by diffing wall times of programs that
    loop the whole body R1 vs R2 times on-device."""
    in_maps = _prep_in_maps(inputs)
    walls = {}
    outs = {}
    for R in loop_reps:
        nc = _build_program(bench_reps=R)
        results, times = _run_pjrt(nc, in_maps, reps=wall_reps)
        walls[R] = min(times)
        out = np.empty((N, 1), np.float32)
        for core in range(NCORES):
            out[core * PER:(core + 1) * PER, 0] = results[core]["out"][:PER]
        outs[R] = out
    R1, R2 = loop_reps
    per_iter = (walls[R2] - walls[R1]) / (R2 - R1)
    return per_iter, walls, outs
